# revision 3
# baseline (speedup 1.0000x reference)
"""EnhancedMultiHeadAttention on 8 Trainium2 NeuronCores (Bass/Tile), v2.

Sharding: core c -> batch b = c//4, head group g = c%4 (4 heads of 16).
Everything is computed in "transposed" layout [feature, token].

Key design points vs v1 (362us):
  - fp8e4m3 DoubleRow matmuls (0.5 cyc/row, 2 K-chunks of 128 per instr)
    for the LN-stat column sums, q/k/gate/v projections, scores and AV:
    PE busy drops ~212us -> ~85us.  The host pre-quantizes x*16 and
    x^2*4 to fp8 so LayerNorm stats need no device elementwise work; the
    mean/bias are folded into the matmuls as a rank-2 correction
    (ncs x mu + b x 1/rstd).
  - The softmax exp (ACT engine, ~133us for S^2 scores: the real floor)
    is software-pipelined under everything else: scores for q-block 0
    are interleaved into the projection phase (probabilities stored as
    fp8, pr = exp(s)*8, written directly by the ACT instruction), and
    the AV/output phases of group g-1 are interleaved into group g's
    score sweep.
  - The 4x41us AllGather of ctx is replaced by out = ReduceScatter(
    ctx_own @ W_out[own rows, :]) per 512-token group (4 collectives of
    21.5us, pipelined behind the exp stream).
  - PSUM: tag "sc" [128,4,256]x2 (scores / po / bc), tag "wk" (2 banks)x2
    (A-phase pqk/pv/stats rotation, post-A ctx accumulators).
All scale factors (fp8 ranges, 1/sqrt(HD), LN rstd, softmax *8) are folded
into host-prepared weights, ACT scale/bias slots, or const lhsT rows, so
the device does no extra scaling work.
"""

import contextlib
import os

import numpy as np
import ml_dtypes

import jax

jax.config.update("jax_compilation_cache_dir", os.path.expanduser("~/.bass_jax_cache"))
jax.config.update("jax_persistent_cache_min_compile_time_secs", 0.0)
jax.config.update("jax_persistent_cache_min_entry_size_bytes", 0)

import concourse.bass as bass
import concourse.bacc as bacc
import concourse.tile as tile
from concourse import mybir
from concourse.bass_utils import run_bass_kernel_spmd
from concourse.hw_specs import get_activation_tables as _orig_gat


def _patched_gat(arch):
    # Steer the greedy ACT-table chooser to the combined ln+exp set so the
    # kernel needs exactly one table load (rstd = exp(-0.5*ln(var+eps))).
    tabs = {k: set(v) for k, v in _orig_gat(arch).items()}
    _AF = mybir.ActivationFunctionType
    for nm in ("exp_and_others", "exp_and_friends"):
        if nm in tabs:
            tabs[nm].discard(_AF.Exp)
    if "natural_log" in tabs:
        tabs["natural_log"].discard(_AF.Ln)
    return tabs


bacc.get_activation_tables = _patched_gat

B, S, D, H, HD = 2, 2048, 1024, 16, 64
NCORES = 8
GROUPS = [[0, 1, 2, 3], [4, 5, 6, 7]]
TB = 512          # token block (A phase) == q group (RS granularity)
NB = S // TB      # 4
FH = 4            # heads per core
FQ = FH * HD      # 256 feature columns per core
NKP = S // 256    # 8 k-chunk pairs (256 tokens contracted per DR matmul)
FP = mybir.dt.float32
FR = mybir.dt.float32r
F16 = mybir.dt.float16
F8 = mybir.dt.float8e4
DR = mybir.MatmulPerfMode.DoubleRow
AF = mybir.ActivationFunctionType
ALU = mybir.AluOpType
EPS = 1e-5

# fp8 scale plan (see numerics.py):
XS = 16.0    # x8 = x * 16            (|x|<5.1 -> <82)
XQS = 4.0    # xq8 = x^2 * 4          (x^2<26 -> <103)
AQ = 32.0    # wq_host = gfold(wq)*32 (folds 1/sqrt(64)*256); qT8 = 256*q
AK = 16.0    # wk_host = gfold(wk)*16; kT8 = 16*k
AG = 32.0    # wg_host = gfold(wg)*32; gu = 32*u (f16)
AV_ = 16.0   # wv_host = gfold(wv)*16; va = 16*v
ESC = 1.0 / 4096.0   # scores_psum = 4096*s
PS = 8.0             # pr = exp(s)*8 (max es ~22.5 -> 180 < 240)
EBIAS = float(np.log(PS))
GESC = -1.0 / 32.0

_NC_CACHE = {}
_BIAS_FREE = [True]


def _body(tc, t):
    nc = tc.nc
    stack = contextlib.ExitStack()
    stack.enter_context(
        nc.allow_low_precision(reason="fp8/f16 rounding is intentional; matmul accumulation stays fp32 in PSUM")
    )
    pool = lambda name, bufs, space="SBUF": stack.enter_context(
        tc.tile_pool(name=name, bufs=bufs, space=space)
    )

    consts = pool("consts", 1)
    singles = pool("singles", 1)

    # PSUM (8 banks): sc 2x(2 banks) | wk 2x(2 banks).
    ps_sc = pool("ps_sc", 2, "PSUM")
    ps_wk = pool("ps_wk", 2, "PSUM")

    # ---- consts -----------------------------------------------------------
    onesf = consts.tile([128, 2, 16], FP)
    nc.vector.memset(onesf, 1.0)
    ones8t = consts.tile([128, 2, 16], F8)
    nc.vector.tensor_copy(out=ones8t, in_=onesf)
    ones8 = ones8t[:, :, 0:1]
    crow_rsb = consts.tile([1, 128], F16)      # rs_b = rstd/XS broadcast
    nc.vector.memset(crow_rsb, 1.0 / XS)
    crow_svf = consts.tile([1, 64], FP)
    nc.vector.memset(crow_svf, 1.0 / AV_)      # ctx descale 1/16
    crow_sv = consts.tile([1, 64], FR)
    nc.vector.tensor_copy(out=crow_sv, in_=crow_svf)
    eps_t = consts.tile([1, 1], FP)
    nc.vector.memset(eps_t, EPS)
    zrow = consts.tile([1, 1], FP)
    nc.vector.memset(zrow, 0.0)
    zcol = consts.tile([128, 1], FP)
    nc.vector.memset(zcol, 0.0)
    ebias = consts.tile([128, 1], FP)
    nc.vector.memset(ebias, EBIAS)

    # ---- resident weights / inputs ---------------------------------------
    w8_sb = singles.tile([128, 4, 2, 3 * FQ], F8)
    wv8_sb = singles.tile([128, 4, 2, FQ], F8)
    wo_sb = singles.tile([64, 4, D], F16)
    cq_sb = singles.tile([2, 3 * FQ], F16)
    cv_sb = singles.tile([2, FQ], F16)
    gc_sb = singles.tile([128, 2], FP)
    bout_sb = singles.tile([128, 2], FP)
    xres_sb = singles.tile([128, 2, S], F16)

    x8_r = t["x8"].ap().rearrange("(kp c p) s -> p kp c s", p=128, c=2)
    xq8_r = t["xq8"].ap().rearrange("(kp c p) s -> p kp c s", p=128, c=2)
    xres_r = t["xres"].ap().rearrange("(m p) s -> p m s", p=128)
    outT_r = t["outT"].ap().rearrange("(m p) s -> p m s", p=128)

    def load_weights():
        g = nc.gpsimd
        g.dma_start(out=w8_sb, in_=t["w8"].ap().rearrange("(kp c p) f -> p kp c f", p=128, c=2))
        g.dma_start(out=cq_sb, in_=t["cq"].ap())
        g.dma_start(out=wv8_sb, in_=t["wv8"].ap().rearrange("(kp c p) f -> p kp c f", p=128, c=2))
        g.dma_start(out=cv_sb, in_=t["cv"].ap())

    def load_weights2():
        g = nc.gpsimd
        g.dma_start(out=wo_sb, in_=t["wo"].ap().rearrange("(kc p) f -> p kc f", p=64))
        g.dma_start(out=gc_sb, in_=t["gc"].ap().rearrange("(m p) -> p m", p=128))
        g.dma_start(out=bout_sb, in_=t["bout"].ap().rearrange("(m p) -> p m", p=128))

    # ---- per-block activations (block == q-group) -------------------------
    qdr = [singles.tile([32, 2, FH, TB], F8, name=f"qdr{i}") for i in range(NB)]
    kdr = [singles.tile([32, 2, FH, TB], F8, name=f"kdr{i}") for i in range(NB)]
    gTs = [singles.tile([128, 2, TB], F16, name=f"gT{i}") for i in range(NB)]
    HDP = 80  # 65 used + pad so the DoubleRow pair stride (4*80) is 16-aligned
    vas = [singles.tile([128, 2, 2, FH, HDP], F8, name=f"va{i}") for i in range(NB)]
    for i in range(NB):
        nc.vector.memset(vas[i], 0.0)
        nc.vector.memset(vas[i][:, :, :, :, HD:HD + 1], 1.0)  # denominator col

    p_x8 = pool("p_x8", 3)        # [128, 4, 2, TB] fp8
    p_xq8 = pool("p_xq8", 2)
    p_strows = pool("p_strows", 2)  # [2, TB] f16: row0 mu, row1 1/rstd
    p_rows = pool("p_rows", 2)    # [1, TB] rows (mu2/var/lnv/rstd)
    p_rsb = pool("p_rsb", 2)      # [128, TB] f16 rstd/XS
    p_rsc = pool("p_rsc", 2)      # [128, 4] f16 rstd cols
    p_qk8 = pool("p_qk8", 2)      # [128, 2, TB] fp8 (q and k tags)
    p_gu = pool("p_gu", 2)        # [128, 2, TB] f16
    p_pr = pool("p_pr", 20)       # [128, 2, FH, 256] fp8 probs (per k-pair, unit)
    p_rden = pool("p_rden", 2)    # [1, FH, 256] FR recip denominators
    p_ctxT = pool("p_ctxT", 2)    # [64, FH, TB] f16
    p_pout = pool("p_pout", 1)    # [128, 8, TB] f16 staging
    p_rso = pool("p_rso", 2)      # [128, 2, TB] f16 RS result
    p_fo16 = pool("p_fo16", 2)    # [128, TB] f16
    p_fo = pool("p_fo", 2)        # [128, 2, TB] f32
    dram_po = pool("dram_po", 3, "DRAM")
    dram_rs = pool("dram_rs", 4, "DRAM")

    x8s, xq8s, strowss, rsbs, rscs = {}, {}, {}, {}, {}
    prs = {}       # (g, u, kp) -> pr tile
    ctxs = {}      # (g, u) -> ctx accumulator
    ctxTs = {}     # g -> ctxT staging tile
    ctxcs = {}     # (g, u) -> unnormalized ctx copy
    parts = {}     # g -> dram partial tile
    pouts_t = {}   # g -> pout staging tile
    rsres = {}     # g -> dram RS result tile
    gts = {}       # g -> finished gate tiles

    def dma_x8(i):
        tb = slice(i * TB, (i + 1) * TB)
        x8 = p_x8.tile([128, 4, 2, TB], F8, tag="x8", name=f"x8_{i}")
        nc.sync.dma_start(out=x8, in_=x8_r[:, :, :, tb])
        x8s[i] = x8

    def dma_xq8(i):
        tb = slice(i * TB, (i + 1) * TB)
        xq = p_xq8.tile([128, 4, 2, TB], F8, tag="xq", name=f"xq{i}")
        nc.sync.dma_start(out=xq, in_=xq8_r[:, :, :, tb])
        xq8s[i] = xq

    def dma_block(i):
        dma_x8(i)
        dma_xq8(i)

    # ---------------- Phase A: LN stats for one token block ---------------
    def stats(i):
        x8 = x8s[i]
        xq = xq8s.pop(i)
        st = ps_wk.tile([1, 2, TB], FP, tag="wk", name=f"st{i}")
        for kp in range(4):
            nc.tensor.matmul(
                out=st[0:1, 0, :], lhsT=ones8, rhs=x8[:, kp, :, :],
                start=(kp == 0), stop=(kp == 3), perf_mode=DR,
            )
        for kp in range(4):
            nc.tensor.matmul(
                out=st[0:1, 1, :], lhsT=ones8, rhs=xq[:, kp, :, :],
                start=(kp == 0), stop=(kp == 3), perf_mode=DR,
            )
        # mu (f16 row, true units), var, rstd = exp(-0.5*ln(var+eps))
        strows = p_strows.tile([2, TB], F16, tag="strows", name=f"strows{i}")
        strowss[i] = strows
        nc.vector.tensor_scalar_mul(
            out=strows[0:1, :], in0=st[0:1, 0, :], scalar1=1.0 / (XS * D)
        )
        mu2 = p_rows.tile([1, TB], FP, tag="row", name=f"mu2{i}")
        nc.vector.tensor_mul(out=mu2, in0=strows[0:1, :], in1=strows[0:1, :])
        var = p_rows.tile([1, TB], FP, tag="row", name=f"var{i}")
        nc.vector.scalar_tensor_tensor(
            out=var, in0=st[0:1, 1, :], scalar=1.0 / (XQS * D), in1=mu2,
            op0=ALU.mult, op1=ALU.subtract,
        )
        lnv = p_rows.tile([1, TB], FP, tag="row", name=f"lnv{i}")
        nc.scalar.activation(out=lnv, in_=var, func=AF.Ln, bias=eps_t[0:1, :])
        rstd = p_rows.tile([1, TB], F16, tag="rstd", name=f"rstd{i}")
        nc.scalar.activation(out=rstd, in_=lnv, func=AF.Exp, scale=-0.5, bias=zrow[0:1, :])
        if not _BIAS_FREE[0]:
            # DVE ops cannot write at partition offset 1; go through a DMA hop
            invr = p_rows.tile([1, TB], F16, tag="invr", name=f"invr{i}")
            nc.vector.reciprocal(out=invr, in_=rstd)
            nc.sync.dma_start(out=strows[1:2, :], in_=invr)
        # broadcast rstd/XS to all 128 partitions
        rb = ps_sc.tile([128, TB], FP, tag="sc", name=f"rb{i}")
        nc.tensor.matmul(out=rb, lhsT=crow_rsb, rhs=rstd, start=True, stop=True)
        rs_b = p_rsb.tile([128, TB], F16, tag="rsb", name=f"rsb{i}")
        nc.vector.tensor_copy(out=rs_b, in_=rb)
        rsbs[i] = rs_b
        rsch = p_rsc.tile([128, 4], F16, tag="rsch", name=f"rsch{i}")
        for mt in range(4):
            nc.sync.dma_start(
                out=rsch[:, mt:mt + 1], in_=rstd[0:1, mt * 128:(mt + 1) * 128]
            )
        rsc = p_rsc.tile([128, 4], FP, tag="rsc", name=f"rsc{i}")
        nc.vector.tensor_copy(out=rsc, in_=rsch)
        rscs[i] = rsc

    # ---------------- Phase A: projections for one token block ------------
    def proj_qkg(i):
        x8 = x8s[i]
        strows = strowss[i]
        rs_b = rsbs[i]
        q8 = p_qk8.tile([128, 2, TB], F8, tag="q8", name=f"q8_{i}")
        k8 = p_qk8.tile([128, 2, TB], F8, tag="k8", name=f"k8_{i}")
        gus = p_gu.tile([128, 2, TB], F16, tag="gu", name=f"gu{i}")
        # q/k/gate in [feat, tok]; mean/bias folded as rank-2 correction
        for m in range(6):
            ms = slice(m * 128, (m + 1) * 128)
            pqk = ps_wk.tile([128, TB], FP, tag="wk", name=f"pqk{i}_{m}")
            for kp in range(4):
                nc.tensor.matmul(
                    out=pqk, lhsT=w8_sb[:, kp, :, ms], rhs=x8[:, kp, :, :],
                    start=(kp == 0), stop=False, perf_mode=DR,
                )
            if _BIAS_FREE[0]:
                nc.tensor.matmul(
                    out=pqk, lhsT=cq_sb[0:1, ms], rhs=strows[0:1, :],
                    start=False, stop=True, skip_group_check=True,
                )
            else:
                nc.tensor.matmul(
                    out=pqk, lhsT=cq_sb[:, ms], rhs=strows,
                    start=False, stop=True, skip_group_check=True,
                )
            if m < 2:
                nc.vector.tensor_mul(out=q8[:, m, :], in0=pqk, in1=rs_b)
                if m == 1:
                    nc.sync.dma_start(out=qdr[i][:, :, 0:4:2, :], in_=q8[0:64, :, :])
                    nc.sync.dma_start(out=qdr[i][:, :, 1:4:2, :], in_=q8[64:128, :, :])
            elif m < 4:
                nc.vector.tensor_mul(out=k8[:, m - 2, :], in0=pqk, in1=rs_b)
                if m == 3:
                    nc.sync.dma_start(out=kdr[i][:, :, 0:4:2, :], in_=k8[0:64, :, :])
                    nc.sync.dma_start(out=kdr[i][:, :, 1:4:2, :], in_=k8[64:128, :, :])
            else:
                nc.vector.tensor_mul(out=gus[:, m - 4, :], in0=pqk, in1=rs_b)
        # gate = 1 / (1 + exp(-u) * gc): only e = exp(-u) here; the cheap
        # DVE finish runs in post() where DVE is otherwise idle
        nc.scalar.activation(out=gTs[i], in_=gus, func=AF.Exp, scale=GESC, bias=zcol[:, 0:1])
    def projv(i):
        x8 = x8s.pop(i)
        strows = strowss.pop(i)
        rsbs.pop(i)
        rsc = rscs.pop(i)
        # v in [tok, feat] (lhsT = x8): va = 16*v
        for mt in range(4):
            tl = slice(mt * 128, (mt + 1) * 128)
            pv = ps_wk.tile([128, FQ], FP, tag="wk", name=f"pv{i}_{mt}")
            for kp in range(4):
                nc.tensor.matmul(
                    out=pv, lhsT=x8[:, kp, :, tl], rhs=wv8_sb[:, kp, :, :],
                    start=(kp == 0), stop=False, perf_mode=DR,
                )
            if _BIAS_FREE[0]:
                nc.tensor.matmul(
                    out=pv, lhsT=strows[0:1, tl], rhs=cv_sb[0:1, :],
                    start=False, stop=True, skip_group_check=True,
                )
            else:
                nc.tensor.matmul(
                    out=pv, lhsT=strows[:, tl], rhs=cv_sb,
                    start=False, stop=True, skip_group_check=True,
                )
            nc.vector.tensor_scalar(
                out=vas[i][:, mt // 2, mt % 2, :, 0:HD], in0=pv,
                scalar1=rsc[:, mt:mt + 1], scalar2=1.0 / XS,
                op0=ALU.mult, op1=ALU.mult,
            )


    # ------------- scores + exp for (group g, unit u, k-pair kp) -----------
    # post-A score PSUM rotates through 2 "sc" slots + 1 "wk" slot (3-deep
    # ACT backlog); during phase A only the 2 "sc" slots are used.
    scup_n = [0]

    def scexp_u(g, u, kp, in_a=False):
        pr = p_pr.tile([128, 2, FH, 256], F8, tag="pr", name=f"pr{g}_{u}_{kp}")
        prs[(g, u, kp)] = pr
        qs = slice(u * 256, (u + 1) * 256)
        for par in range(2):
            kc = 2 * kp + par
            kb, kl = divmod(kc, 4)
            ks_ = slice(kl * 128, (kl + 1) * 128)
            scup_n[0] += 1
            if in_a or scup_n[0] % 3:
                sc = ps_sc.tile([128, FH, 256], FP, tag="sc", name=f"sc{g}_{u}_{kp}_{par}")
            else:
                sc = ps_wk.tile([128, FH, 256], FP, tag="wk", name=f"sc{g}_{u}_{kp}_{par}")
            for h in range(FH):
                nc.tensor.matmul(
                    out=sc[:, h, :], lhsT=kdr[kb][:, :, h, ks_],
                    rhs=qdr[g][:, :, h, qs],
                    start=True, stop=True, perf_mode=DR, skip_group_check=True,
                )
            nc.scalar.activation(
                out=pr[:, par, :, :], in_=sc,
                func=AF.Exp, scale=ESC, bias=ebias[:, 0:1],
            )

    # ---------------- AV accumulation for (g, u, kp) ------------------------
    def alloc_ctx(g, u):
        ctxs[(g, u)] = ps_wk.tile(
            [80, FH, 256], FP, tag="wk", name=f"ctx{g}_{u}"
        )

    def av_u(g, u, kp):
        pr = prs.pop((g, u, kp))
        kb, kpl = divmod(kp, 2)
        for h in range(FH):
            nc.tensor.matmul(
                out=ctxs[(g, u)][:, h, :],
                lhsT=vas[kb][:, kpl, :, h, :],
                rhs=pr[:, :, h, :],
                start=(kp == 0), stop=(kp == NKP - 1), perf_mode=DR,
                skip_group_check=True,
            )

    # -------- evac one ctx unit (normalize to f16) + its out-proj half -----
    def evac_a(g, u):
        # unnormalized ctx copy can start as soon as AV is done (PSUM->SBUF),
        # overlapping the reciprocal/broadcast of the denominators
        ctxc = p_rden.tile([64, FH, 256], F16, tag="ctxc", name=f"ctxc{g}_{u}")
        nc.vector.tensor_copy(out=ctxc, in_=ctxs[(g, u)][0:HD, :, :])
        ctxcs[(g, u)] = ctxc

    def evac_b(g, u):
        if g not in ctxTs:
            ctxTs[g] = p_ctxT.tile([64, FH, TB], F16, tag="ctxT", name=f"ctxT{g}")
        ctxT = ctxTs[g]
        cu = ctxs.pop((g, u))
        ctxc = ctxcs.pop((g, u))
        us = slice(u * 256, (u + 1) * 256)
        rden = p_rden.tile([1, FH, 256], FR, tag="rden", name=f"rden{g}_{u}")
        for h in range(FH):
            nc.vector.reciprocal(out=rden[0:1, h, :], in_=cu[HD:HD + 1, h, :])
        bc = ps_sc.tile([64, FH, 256], FP, tag="sc", name=f"bc{g}_{u}")
        for h in range(FH):
            nc.tensor.matmul(
                out=bc[:, h, :], lhsT=crow_sv, rhs=rden[0:1, h, :],
                start=True, stop=True, skip_group_check=True,
            )
        nc.vector.tensor_mul(out=ctxT[:, :, us], in0=ctxc, in1=bc)
        if u == 1:
            ctxTs.pop(g)
        return ctxT

    ctxTs2 = {}

    def evac_u(g, u):
        evac_a(g, u)
        ctxTs2[(g, u)] = evac_b(g, u)

    def po_u(g, u, mp):
        # out-proj (K=64 chunks read ctxT directly), one pair of m-tiles
        ctxT = ctxTs2[(g, u)]
        us = slice(u * 256, (u + 1) * 256)
        if g not in parts:
            parts[g] = dram_po.tile([D, TB], F16, tag="part", name=f"part{g}")
            pouts_t[g] = p_pout.tile([128, 8, TB], F16, tag="pout", name=f"pout{g}")
        pouts = pouts_t[g]
        po = ps_sc.tile([128, 2, 256], FP, tag="sc", name=f"po{g}_{u}_{mp}")
        for m2 in range(2):
            m = 2 * mp + m2
            for h in range(FH):
                nc.tensor.matmul(
                    out=po[:, m2, :], lhsT=wo_sb[:, h, m * 128:(m + 1) * 128],
                    rhs=ctxT[:, h, us], start=(h == 0), stop=(h == FH - 1),
                    skip_group_check=True,
                )
        nc.vector.tensor_copy(out=pouts[:, 2 * mp:2 * mp + 2, us], in_=po)

    def part_half(g, u):
        ctxTs2.pop((g, u))
        us = slice(u * 256, (u + 1) * 256)
        nc.sync.dma_start(
            out=parts[g].rearrange("(m p) s -> p m s", p=128)[:, :, us],
            in_=pouts_t[g][:, :, us],
        )

    def cphase(g):
        gts[g] = []
        for m in range(2):
            gp = p_fo16.tile([128, TB], F16, tag="gp", name=f"gp{g}_{m}")
            nc.vector.tensor_scalar(
                out=gp, in0=gTs[g][:, m, :], scalar1=gc_sb[:, m:m + 1], scalar2=1.0,
                op0=ALU.mult, op1=ALU.add,
            )
            gt = p_fo16.tile([128, TB], F16, tag="gt", name=f"gt{g}_{m}")
            nc.vector.reciprocal(out=gt, in_=gp)
            gts[g].append(gt)
        parts_g = parts.pop(g)
        pouts_t.pop(g)
        rsr = dram_rs.tile([2, 128, TB], F16, tag="rsr", name=f"rsr{g}")
        rsres[g] = rsr
        nc.gpsimd.collective_compute(
            "ReduceScatter",
            ALU.add,
            replica_groups=GROUPS,
            ins=[parts_g.opt()],
            outs=[rsr.opt()],
        )

    # ---------------- post: bias + gate + residual + store -----------------
    def post(g):
        qs = slice(g * TB, (g + 1) * TB)
        rsr = rsres.pop(g)
        rso = p_rso.tile([128, 2, TB], F16, tag="rso", name=f"rso{g}")
        fo = p_fo.tile([128, 2, TB], FP, tag="fo", name=f"fo{g}")
        for m in range(2):
            nc.sync.dma_start(out=rso[:, m, :], in_=rsr[m, :, :])
            f16t = p_fo16.tile([128, TB], F16, tag="fo16", name=f"fo16{g}_{m}")
            nc.vector.scalar_tensor_tensor(
                out=f16t, in0=rso[:, m, :], scalar=bout_sb[:, m:m + 1],
                in1=gts[g][m], op0=ALU.add, op1=ALU.mult,
            )
            nc.vector.tensor_add(out=fo[:, m, :], in0=f16t, in1=xres_sb[:, m, qs])
            nc.sync.dma_start(out=outT_r[:, m, qs], in_=fo[:, m, :])
        gts.pop(g)

    # ======================= emission schedule =============================
    av_next = {}
    emitted = {}   # (g, u) -> number of scexp kps emitted
    dma_x8(0)
    load_weights()
    dma_xq8(0)
    dma_block(1)
    stats(0)
    proj_qkg(0)
    for i in range(NB):
        if i + 1 < NB:
            stats(i + 1)
        for u in range(2):
            scexp_u(0, u, 2 * i, in_a=True)
        projv(i)
        for u in range(2):
            scexp_u(0, u, 2 * i + 1, in_a=True)
        if i + 1 < NB:
            proj_qkg(i + 1)
        if i + 2 < NB:
            dma_block(i + 2)
        if i == 0:
            load_weights2()
            nc.sync.dma_start(out=xres_sb, in_=xres_r)

    # post-A: one 16-exp score sweep per (group, unit); ctx accumulation of
    # the previous sweep's prs and the evac/C/RS chain are interleaved at
    # fixed kp milestones to keep the exp stream fed.
    sweeps = [(g, u) for g in range(1, NB) for u in range(2)]

    def pump(g, u, n):
        j = av_next[(g, u)]
        while j < min(av_next[(g, u)] + n, NKP):
            av_u(g, u, j)
            j += 1
        av_next[(g, u)] = j

    def start_ctx(g, u):
        alloc_ctx(g, u)
        av_next[(g, u)] = 0

    for si, (g, u) in enumerate(sweeps):
        p = ((g, u - 1) if u else (g - 1, 1))  # previous sweep
        for kp in range(NKP):
            scexp_u(g, u, kp)
            if si == 0:
                # group-0 AV/evac/out-proj (prs stored during phase A)
                if kp == 0:
                    start_ctx(0, 0)
                    pump(0, 0, 4)
                elif kp == 1:
                    pump(0, 0, 4)
                elif kp == 2:
                    evac_a(0, 0)
                elif kp == 3:
                    ctxTs2[(0, 0)] = evac_b(0, 0)
                elif kp == 4:
                    start_ctx(0, 1)
                    pump(0, 1, 4)
                    po_u(0, 0, 0)
                    po_u(0, 0, 1)
                elif kp == 5:
                    pump(0, 1, 4)
                    po_u(0, 0, 2)
                    po_u(0, 0, 3)
                    part_half(0, 0)
                elif kp == 6:
                    evac_a(0, 1)
                elif kp == 7:
                    ctxTs2[(0, 1)] = evac_b(0, 1)
            elif si == 1:
                if kp == 0:
                    start_ctx(*p)
                    pump(*p, 2)
                elif kp == 1:
                    pump(*p, NKP)
                    evac_a(*p)
                elif kp == 2:
                    ctxTs2[p] = evac_b(*p)
                elif kp == 3:
                    start_ctx(g, u)
                    pump(g, u, 1)
                    po_u(0, 1, 0)
                    po_u(0, 1, 1)
                elif kp == 4:
                    pump(g, u, 1)
                    po_u(0, 1, 2)
                    po_u(0, 1, 3)
                    part_half(0, 1)
                    cphase(0)
                elif kp == 5:
                    pump(g, u, 1)
                    po_u(*p, 0)
                elif kp == 6:
                    pump(g, u, 1)
                    po_u(*p, 1)
                    po_u(*p, 2)
                elif kp == 7:
                    pump(g, u, 1)
                    po_u(*p, 3)
                    part_half(*p)
            else:
                if kp == 0:
                    pump(*p, 2)
                elif kp == 1:
                    pump(*p, NKP)
                    evac_a(*p)
                elif kp == 2:
                    ctxTs2[p] = evac_b(*p)
                elif kp == 3:
                    start_ctx(g, u)
                    pump(g, u, 1)
                    po_u(*p, 0)
                elif kp == 4:
                    pump(g, u, 1)
                    po_u(*p, 1)
                elif kp == 5:
                    pump(g, u, 1)
                    po_u(*p, 2)
                elif kp == 6:
                    pump(g, u, 1)
                    po_u(*p, 3)
                    part_half(*p)
                    if p[1] == 1:
                        cphase(p[0])
                elif kp == 7:
                    pump(g, u, 1)
                    if p[1] == 1 and p[0] - 2 >= 0:
                        post(p[0] - 2)
    # tail: last sweep's AV remainder + finish
    g, u = sweeps[-1]
    pump(g, u, NKP)
    evac_u(g, u)
    for mp in range(4):
        po_u(g, u, mp)
    part_half(g, u)
    cphase(g)
    post(NB - 3)
    post(NB - 2)
    post(NB - 1)

    stack.close()


def build_nc():
    if "nc" in _NC_CACHE:
        return _NC_CACHE["nc"]
    nc = bacc.Bacc("TRN2", target_bir_lowering=False, debug=False, num_devices=NCORES)
    t = {}
    t["x8"] = nc.dram_tensor("x8", [D, S], F8, kind="ExternalInput")
    t["xq8"] = nc.dram_tensor("xq8", [D, S], F8, kind="ExternalInput")
    t["xres"] = nc.dram_tensor("xres", [FQ, S], F16, kind="ExternalInput")
    t["w8"] = nc.dram_tensor("w8", [D, 3 * FQ], F8, kind="ExternalInput")
    t["wv8"] = nc.dram_tensor("wv8", [D, FQ], F8, kind="ExternalInput")
    t["wo"] = nc.dram_tensor("wo", [FQ, D], F16, kind="ExternalInput")
    t["cq"] = nc.dram_tensor("cq", [2, 3 * FQ], F16, kind="ExternalInput")
    t["cv"] = nc.dram_tensor("cv", [2, FQ], F16, kind="ExternalInput")
    t["gc"] = nc.dram_tensor("gc", [FQ], FP, kind="ExternalInput")
    t["bout"] = nc.dram_tensor("bout", [FQ], FP, kind="ExternalInput")
    t["outT"] = nc.dram_tensor("outT", [FQ, S], FP, kind="ExternalOutput")
    with tile.TileContext(nc) as tc:
        _body(tc, t)
    nc.finalize()
    _NC_CACHE["nc"] = nc
    return nc


E4NP = ml_dtypes.float8_e4m3


def _q8(a):
    return np.asarray(a, np.float32).astype(E4NP)


def make_in_maps(x, gamma, beta, w_qkv, b_qkv, w_out, b_out, w_gate, b_gate):
    x = np.asarray(x, np.float32)
    gamma = np.asarray(gamma, np.float32)
    beta = np.asarray(beta, np.float32)
    w_qkv = np.asarray(w_qkv, np.float32)
    b_qkv = np.asarray(b_qkv, np.float32)
    w_out = np.asarray(w_out, np.float32)
    b_out = np.asarray(b_out, np.float32)
    w_gate = np.asarray(w_gate, np.float32)
    b_gate = np.asarray(b_gate, np.float32)

    xT = [np.ascontiguousarray(x[b].T) for b in range(B)]
    x8T = [_q8(a * XS) for a in xT]
    xq8T = [_q8((a * a) * XQS) for a in xT]

    gfold = lambda w: gamma[:, None] * w
    bfold = lambda w, bb: bb + beta @ w
    _BIAS_FREE[0] = bool(
        np.all(b_qkv == 0) and np.all(beta == 0)
    )

    in_maps = []
    for c in range(NCORES):
        b, g = divmod(c, 4)
        cols = slice(g * FQ, (g + 1) * FQ)
        wq = gfold(w_qkv[:, 0 * D:1 * D][:, cols])
        wk = gfold(w_qkv[:, 1 * D:2 * D][:, cols])
        wv = gfold(w_qkv[:, 2 * D:3 * D][:, cols])
        wg = gfold(w_gate[:, cols])
        bq = bfold(w_qkv[:, 0 * D:1 * D][:, cols], b_qkv[0 * D:1 * D][cols])
        bk = bfold(w_qkv[:, 1 * D:2 * D][:, cols], b_qkv[1 * D:2 * D][cols])
        bv = bfold(w_qkv[:, 2 * D:3 * D][:, cols], b_qkv[2 * D:3 * D][cols])
        bg = bfold(w_gate[:, cols], b_gate[cols])

        w8 = np.ascontiguousarray(np.concatenate(
            [_q8(wq * AQ), _q8(wk * AK), _q8(wg * AG)], axis=1))
        wv8 = np.ascontiguousarray(_q8(wv * AV_))
        w8f = w8.astype(np.float32)
        wv8f = wv8.astype(np.float32)
        # corr rows: row0 x mu (mean removal), row1 x (1/rstd) (bias)
        cq = np.stack([
            -XS * w8f.sum(axis=0),
            np.concatenate([512.0 * bq, 256.0 * bk, np.zeros(FQ, np.float32)]),
        ]).astype(np.float16)
        cv = np.stack([
            -XS * wv8f.sum(axis=0),
            256.0 * bv,
        ]).astype(np.float16)

        in_maps.append({
            "x8": x8T[b],
            "xq8": xq8T[b],
            "xres": xT[b][cols, :].astype(np.float16),
            "w8": w8,
            "wv8": wv8,
            "wo": np.ascontiguousarray(w_out[cols, :]).astype(np.float16),
            "cq": np.ascontiguousarray(cq),
            "cv": np.ascontiguousarray(cv),
            "gc": np.exp(-bg).astype(np.float32),
            "bout": np.ascontiguousarray(b_out[cols]).astype(np.float32),
        })
    return in_maps


def run_device(in_maps):
    nc = build_nc()
    return run_bass_kernel_spmd(nc, in_maps, list(range(NCORES)))


def assemble(results):
    out = np.empty((B, S, D), np.float32)
    for c in range(NCORES):
        b, g = divmod(c, 4)
        out[b][:, g * FQ:(g + 1) * FQ] = results[c]["outT"].T
    return out


def kernel(**inputs):
    in_maps = make_in_maps(**inputs)
    res = run_device(in_maps)
    return assemble(res.results)
